# revision 4
# baseline (speedup 1.0000x reference)
"""Trainium2 Bass kernel for nn_AttentionFusion (dense_transformer).

Reference computation per batch element b (B=8 -> one NeuronCore each):
    w_ds = bilinear_downsample(feat_wide[b], 4)   # [C,64,64], exact 2x2 avg at (4i+1..4i+2)
    n_ds = bilinear_downsample(feat_narrow[b], 4)
    Q = w_ds.reshape(C, N); K = n_ds.reshape(C, N)    # N = 4096
    attn = softmax(Q^T K / sqrt(C), axis=-1)          # [N, N]
    out_small = (attn @ K^T)^T                        # [C, N]
    out = feat_wide[b] + bilinear_upsample(out_small.reshape(C,64,64), 4)

Mapping to the hardware (all on-chip after the DMA loads):
  - downsample: strided DMA of rows 4i+1,4i+2 only + DVE adds -> q_bf/k_bf bf16 [128, 4096]
  - K^T (+ ones column for the softmax row-sum) via PE transposes -> kt1 [128, 32, 129]
  - scores^T computed per (m-tile, n-block): PE matmul lhsT=K-tile rhs=Q-block
  - exp on ScalarE (scale=1/sqrt(C) folded in), written as bf16 attn^T tiles
  - PV: PE matmul lhsT=attnT-tile rhs=kt1-tile accumulating over m; the extra
    ones column yields the softmax denominator per partition; normalize with
    DVE reciprocal + tensor_scalar
  - W-upsample: PE matmul with a constant [128, 512] block-diagonal weight
  - H-upsample + residual: DVE scalar_tensor_tensor pairs, streamed per 16 rows
"""

import math

import numpy as np


# ----------------------------------------------------------------------------
# numpy-side constants
# ----------------------------------------------------------------------------

def _build_upsample_matrix(n_in: int, n_out: int) -> np.ndarray:
    """U[h, H]: out[H] = sum_h U[h, H] * in[h] for torch-style bilinear,
    align_corners=False, antialias=False, scale n_out/n_in."""
    U = np.zeros((n_in, n_out), dtype=np.float64)
    scale = n_in / n_out
    for o in range(n_out):
        src = (o + 0.5) * scale - 0.5
        k0 = int(math.floor(src))
        frac = src - k0
        for k, wt in ((k0, 1.0 - frac), (k0 + 1, frac)):
            kc = min(max(k, 0), n_in - 1)
            U[kc, o] += wt
    return U


def _build_uw_block() -> np.ndarray:
    """[128, 512] block-diag W-upsample weights: two 64->256 blocks."""
    U = _build_upsample_matrix(64, 256)
    blk = np.zeros((128, 512), dtype=np.float64)
    blk[0:64, 0:256] = U
    blk[64:128, 256:512] = U
    return blk


# ----------------------------------------------------------------------------
# Bass kernel builder
# ----------------------------------------------------------------------------

def build_kernel():
    import concourse.bacc as bacc
    import concourse.bass as bass
    import concourse.mybir as mybir
    from concourse import tile

    f32 = mybir.dt.float32
    bf16 = mybir.dt.bfloat16
    AOp = mybir.AluOpType
    ActFn = mybir.ActivationFunctionType

    C = 128          # channels = partitions
    HW = 256         # full resolution
    hw = 64          # downsampled resolution
    N = hw * hw      # 4096 attention positions
    MI = 32          # m tiles of 128
    NBLK = 8         # n blocks of 512
    NSUB = 4         # n sub-tiles of 128 per block
    INV_SQRT_C = 1.0 / math.sqrt(C)

    nc = bacc.Bacc("TRN2", target_bir_lowering=False, debug=False)

    fw = nc.dram_tensor("feat_wide", [C, HW, HW], f32, kind="ExternalInput")
    fn = nc.dram_tensor("feat_narrow", [C, HW, HW], f32, kind="ExternalInput")
    uw = nc.dram_tensor("uwblk", [128, 512], bf16, kind="ExternalInput")
    ident = nc.dram_tensor("ident", [128, 128], bf16, kind="ExternalInput")
    out = nc.dram_tensor("out", [C, HW, HW], f32, kind="ExternalOutput")

    with tile.TileContext(nc) as tc:
        with (
            tc.tile_pool(name="const", bufs=1) as const_pool,
            tc.tile_pool(name="qk", bufs=1) as qk_pool,
            tc.tile_pool(name="ds", bufs=2) as ds_pool,
            tc.tile_pool(name="io", bufs=2) as io_pool,
            tc.tile_pool(name="attn", bufs=2) as attn_pool,
            tc.tile_pool(name="small", bufs=3) as small_pool,
            tc.tile_pool(name="dstmp", bufs=3) as dstmp_pool,
            tc.tile_pool(name="ps_s", bufs=2, space=bass.MemorySpace.PSUM) as ps_s,
            tc.tile_pool(name="ps_o", bufs=2, space=bass.MemorySpace.PSUM) as ps_o,
            tc.tile_pool(name="ps_y", bufs=1, space=bass.MemorySpace.PSUM) as ps_y,
            tc.tile_pool(name="ps_t", bufs=1, space=bass.MemorySpace.PSUM) as ps_t,
        ):
            # ---- constants ----
            uw_t = const_pool.tile([128, 512], bf16)
            nc.sync.dma_start(uw_t[:], uw[:, :])
            id_t = const_pool.tile([128, 128], bf16)
            nc.sync.dma_start(id_t[:], ident[:, :])

            # ---- downsample: feat -> q_bf / k_bf  [128, 4096] bf16 ----
            q_bf = qk_pool.tile([C, N], bf16)
            k_bf = qk_pool.tile([C, N], bf16)

            DS_I = 4  # i-rows per chunk
            for src, dst in ((fw, q_bf), (fn, k_bf)):
                src3 = src.ap().rearrange("c (i r) w -> c i (r w)", r=4)  # [128,64,1024]
                dst3 = dst[:].rearrange("c (i w) -> c i w", w=hw)          # [128,64,64]
                for cc in range(hw // DS_I):
                    i0 = cc * DS_I
                    ch = ds_pool.tile([C, DS_I, 512], f32, tag="ch")
                    # rows 4i+1, 4i+2 are adjacent -> 2KB contiguous lines
                    nc.sync.dma_start(ch[:], src3[:, i0 : i0 + DS_I, 256:768])
                    ch5 = ch[:].rearrange("c i (r k f) -> c i r k f", r=2, f=4)
                    # ds = 0.25*(x[4i+1,4j+1]+x[4i+1,4j+2]+x[4i+2,4j+1]+x[4i+2,4j+2])
                    a = dstmp_pool.tile([C, DS_I, hw], f32, tag="dsa")
                    b = dstmp_pool.tile([C, DS_I, hw], f32, tag="dsb")
                    nc.vector.tensor_tensor(
                        a[:], ch5[:, :, 0, :, 1], ch5[:, :, 1, :, 1], AOp.add
                    )
                    nc.vector.tensor_tensor(
                        b[:], ch5[:, :, 0, :, 2], ch5[:, :, 1, :, 2], AOp.add
                    )
                    c_t = dstmp_pool.tile([C, DS_I, hw], f32, tag="dsc")
                    nc.vector.tensor_tensor(c_t[:], a[:], b[:], AOp.add)
                    nc.vector.tensor_scalar(
                        dst3[:, i0 : i0 + DS_I, :], c_t[:], 0.25, None, AOp.mult
                    )

            # ---- K^T with ones column: kt1 [128(m), 32(mi), 129] bf16 ----
            kt1 = qk_pool.tile([128, MI, 129], bf16)
            nc.vector.memset(kt1[:, :, 128], 1.0)
            for mi in range(MI):
                pt = ps_t.tile([128, 128], bf16)
                nc.tensor.transpose(pt[:], k_bf[:, mi * 128 : (mi + 1) * 128], id_t[:])
                nc.vector.tensor_copy(kt1[:, mi, 0:128], pt[:])

            # ---- y = W-upsampled attention output [128, 64, 256] bf16 ----
            y = qk_pool.tile([C, hw, HW], bf16)
            y3 = y[:]  # [128, 64, 256]

            # ---- attention over n-blocks ----
            for nb in range(NBLK):
                at = attn_pool.tile([128, MI, 512], bf16, tag="at")
                for mp in range(MI // 2):
                    ps = ps_s.tile([128, 2, 512], f32)
                    for s in range(2):
                        mi = 2 * mp + s
                        nc.tensor.matmul(
                            ps[:, s, :],
                            k_bf[:, mi * 128 : (mi + 1) * 128],
                            q_bf[:, nb * 512 : (nb + 1) * 512],
                            start=True,
                            stop=True,
                        )
                    # attnT = exp(scoresT / sqrt(C)), bf16
                    nc.scalar.activation(
                        at[:, 2 * mp : 2 * mp + 2, :],
                        ps[:],
                        ActFn.Exp,
                        bias=0.0,
                        scale=INV_SQRT_C,
                    )
                for ns in range(NSUB):
                    t = nb * NSUB + ns  # global n-tile (2 h-rows)
                    po = ps_o.tile([128, 129], f32)
                    for mi in range(MI):
                        nc.tensor.matmul(
                            po[:],
                            at[:, mi, ns * 128 : (ns + 1) * 128],
                            kt1[:, mi, :],
                            start=(mi == 0),
                            stop=(mi == MI - 1),
                        )
                    rcp = small_pool.tile([128, 1], f32, tag="rcp")
                    nc.vector.reciprocal(rcp[:], po[:, 128:129])
                    ot = small_pool.tile([128, 128], bf16, tag="ot")
                    nc.vector.tensor_scalar(ot[:], po[:, 0:128], rcp[:], None, AOp.mult)
                    # W-upsample two rows: py[c, (h2, W)]
                    py = ps_y.tile([128, 512], f32)
                    nc.tensor.matmul(py[:], ot[:], uw_t[:], start=True, stop=True)
                    nc.scalar.copy(y3[:, 2 * t : 2 * t + 2, :], py[:])

            # ---- H-upsample + residual, streamed in blocks of 16 output rows ----
            # out[4k+r] = wa[r]*y[k+d[r]] + wb[r]*y[k+d[r]+1] + fw[4k+r]
            # r=0: 0.375*y[k-1] + 0.625*y[k]
            # r=1: 0.125*y[k-1] + 0.875*y[k]
            # r=2: 0.875*y[k]   + 0.125*y[k+1]
            # r=3: 0.625*y[k]   + 0.375*y[k+1]
            PH = (
                (0.375, 0.625, -1),
                (0.125, 0.875, -1),
                (0.875, 0.125, 0),
                (0.625, 0.375, 0),
            )
            KB = 4  # y rows per block -> 16 output rows
            for kb in range(hw // KB):
                k0 = kb * KB
                h0 = 4 * k0
                fwb = io_pool.tile([C, 4 * KB, HW], f32, tag="io")
                nc.sync.dma_start(fwb[:], fw.ap()[:, h0 : h0 + 4 * KB, :])
                ob = io_pool.tile([C, 4 * KB, HW], f32, tag="io")
                fw4 = fwb[:].rearrange("c (j r) w -> c j r w", r=4)
                ob4 = ob[:].rearrange("c (j r) w -> c j r w", r=4)
                for r, (wa, wb, d) in enumerate(PH):
                    # rows j where k0+j+d and k0+j+d+1 are both in [0, 63]
                    js, je = 0, KB
                    if kb == 0 and d == -1:
                        js = 1
                    if kb == hw // KB - 1 and d == 0:
                        je = KB - 1
                    # edge rows: clamped -> out = 1.0*y[edge] + fw
                    if js == 1:
                        nc.vector.scalar_tensor_tensor(
                            ob4[:, 0, r, :], y3[:, 0, :], 1.0,
                            fw4[:, 0, r, :], AOp.mult, AOp.add,
                        )
                    if je == KB - 1:
                        nc.vector.scalar_tensor_tensor(
                            ob4[:, KB - 1, r, :], y3[:, hw - 1, :], 1.0,
                            fw4[:, KB - 1, r, :], AOp.mult, AOp.add,
                        )
                    cnt = je - js
                    ka = k0 + js + d
                    tm = small_pool.tile([C, KB, HW], bf16, tag="tm")
                    # tm = (wa/wb) * y[ka..] + y[ka+1..]       (all bf16, 2x mode)
                    nc.vector.scalar_tensor_tensor(
                        tm[:, 0:cnt, :], y3[:, ka : ka + cnt, :], wa / wb,
                        y3[:, ka + 1 : ka + 1 + cnt, :], AOp.mult, AOp.add,
                    )
                    # out = wb * tm + fw
                    nc.vector.scalar_tensor_tensor(
                        ob4[:, js:je, r, :], tm[:, 0:cnt, :], wb,
                        fw4[:, js:je, r, :], AOp.mult, AOp.add,
                    )
                nc.sync.dma_start(out.ap()[:, h0 : h0 + 4 * KB, :], ob[:])

    nc.compile()
    return nc


_NC_CACHE = None


def _get_nc():
    global _NC_CACHE
    if _NC_CACHE is None:
        _NC_CACHE = build_kernel()
    return _NC_CACHE


def run(feat_wide: np.ndarray, feat_narrow: np.ndarray, trace: bool = False):
    """Run on 8 NeuronCores; returns (output [8,128,256,256], BassKernelResults)."""
    from concourse.bass_utils import run_bass_kernel_spmd
    import ml_dtypes

    B, C, H, W = feat_wide.shape
    assert (B, C, H, W) == (8, 128, 256, 256)

    uwblk = _build_uw_block().astype(ml_dtypes.bfloat16)
    identity = np.eye(128, dtype=ml_dtypes.bfloat16)

    nc = _get_nc()
    in_maps = [
        {
            "feat_wide": np.ascontiguousarray(np.asarray(feat_wide[b], dtype=np.float32)),
            "feat_narrow": np.ascontiguousarray(np.asarray(feat_narrow[b], dtype=np.float32)),
            "uwblk": uwblk,
            "ident": identity,
        }
        for b in range(B)
    ]
    res = run_bass_kernel_spmd(nc, in_maps, core_ids=list(range(8)), trace=trace)
    out = np.stack([res.results[b]["out"] for b in range(B)], axis=0)
    return out, res


def kernel(feat_wide: np.ndarray, feat_narrow: np.ndarray) -> np.ndarray:
    out, _ = run(feat_wide, feat_narrow, trace=False)
    return out


# revision 6
# speedup vs baseline: 1.3049x; 1.3049x over previous
"""Trainium2 Bass kernel for nn_AttentionFusion (dense_transformer).

Reference computation per batch element b (B=8 -> one NeuronCore each):
    w_ds = bilinear_downsample(feat_wide[b], 4)   # [C,64,64], exact 2x2 avg at (4i+1..4i+2)
    n_ds = bilinear_downsample(feat_narrow[b], 4)
    Q = w_ds.reshape(C, N); K = n_ds.reshape(C, N)    # N = 4096
    attn = softmax(Q^T K / sqrt(C), axis=-1)          # [N, N]
    out_small = (attn @ K^T)^T                        # [C, N]
    out = feat_wide[b] + bilinear_upsample(out_small.reshape(C,64,64), 4)

Mapping to the hardware (all on-chip after the DMA loads):
  - downsample: strided DMA of rows 4i+1,4i+2 only + DVE adds -> q_bf/k_bf bf16
    [128, 4096]; the 0.25 average scale is folded into the exp scale (1/16) and
    the W-upsample constant (1/4), so the adds are unscaled.
  - K^T (+ ones column for the softmax row-sum) via PE transposes -> kt1
  - scores^T per (m-tile, n-block): PE matmul lhsT=K-tile rhs=Q-block; the PV
    matmuls of the previous n-block are interleaved in program order so the PE
    stays dense while ScalarE runs the exps.
  - exp on ScalarE (scale folded in), written as bf16 attn^T tiles
  - PV: PE matmul lhsT=attnT-tile rhs=kt1-tile accumulating over m; the ones
    column yields the softmax denominator per partition; normalize with DVE
    reciprocal + tensor_scalar
  - W-upsample: PE matmul with a constant [128, 512] block-diagonal weight
  - H-upsample + residual: DVE scalar_tensor_tensor pairs, computed in place
    on the streamed feat_wide row blocks
"""

import math

import numpy as np


# ----------------------------------------------------------------------------
# numpy-side constants
# ----------------------------------------------------------------------------

def _build_upsample_matrix(n_in: int, n_out: int) -> np.ndarray:
    """U[h, H]: out[H] = sum_h U[h, H] * in[h] for torch-style bilinear,
    align_corners=False, antialias=False, scale n_out/n_in."""
    U = np.zeros((n_in, n_out), dtype=np.float64)
    scale = n_in / n_out
    for o in range(n_out):
        src = (o + 0.5) * scale - 0.5
        k0 = int(math.floor(src))
        frac = src - k0
        for k, wt in ((k0, 1.0 - frac), (k0 + 1, frac)):
            kc = min(max(k, 0), n_in - 1)
            U[kc, o] += wt
    return U


def _build_uw_block() -> np.ndarray:
    """[128, 512] block-diag W-upsample weights (two 64->256 blocks), pre-scaled
    by 1/4 to undo the unscaled 2x2-average downsample of K."""
    U = _build_upsample_matrix(64, 256) * 0.25
    blk = np.zeros((128, 512), dtype=np.float64)
    blk[0:64, 0:256] = U
    blk[64:128, 256:512] = U
    return blk


# ----------------------------------------------------------------------------
# Bass kernel builder
# ----------------------------------------------------------------------------

def build_kernel():
    import concourse.bacc as bacc
    import concourse.bass as bass
    import concourse.mybir as mybir
    from concourse import tile

    f32 = mybir.dt.float32
    bf16 = mybir.dt.bfloat16
    AOp = mybir.AluOpType
    ActFn = mybir.ActivationFunctionType

    C = 128          # channels = partitions
    HW = 256         # full resolution
    hw = 64          # downsampled resolution
    N = hw * hw      # 4096 attention positions
    MI = 32          # m tiles of 128
    NBLK = 8         # n blocks of 512
    NSUB = 4         # n sub-tiles of 128 per block
    # scores = (4Q)^T (4K) / (16 sqrt(C)); the ds 2x2 sums are unscaled
    EXP_SCALE = 1.0 / (16.0 * math.sqrt(C))

    nc = bacc.Bacc("TRN2", target_bir_lowering=False, debug=False)

    fw = nc.dram_tensor("feat_wide", [C, HW, HW], f32, kind="ExternalInput")
    fn = nc.dram_tensor("feat_narrow", [C, HW, HW], f32, kind="ExternalInput")
    uw = nc.dram_tensor("uwblk", [128, 512], bf16, kind="ExternalInput")
    ident = nc.dram_tensor("ident", [128, 128], bf16, kind="ExternalInput")
    out = nc.dram_tensor("out", [C, HW, HW], f32, kind="ExternalOutput")

    with tile.TileContext(nc) as tc:
        with (
            tc.tile_pool(name="const", bufs=1) as const_pool,
            tc.tile_pool(name="qk", bufs=1) as qk_pool,
            tc.tile_pool(name="ds", bufs=2) as ds_pool,
            tc.tile_pool(name="io", bufs=3) as io_pool,
            tc.tile_pool(name="attn", bufs=2) as attn_pool,
            tc.tile_pool(name="small", bufs=3) as small_pool,
            tc.tile_pool(name="dstmp", bufs=3) as dstmp_pool,
            tc.tile_pool(name="ps_s", bufs=2, space=bass.MemorySpace.PSUM) as ps_s,
            tc.tile_pool(name="ps_o", bufs=2, space=bass.MemorySpace.PSUM) as ps_o,
            tc.tile_pool(name="ps_y", bufs=1, space=bass.MemorySpace.PSUM) as ps_y,
            tc.tile_pool(name="ps_t", bufs=1, space=bass.MemorySpace.PSUM) as ps_t,
        ):
            # ---- constants ----
            uw_t = const_pool.tile([128, 512], bf16)
            nc.sync.dma_start(uw_t[:], uw[:, :])
            id_t = const_pool.tile([128, 128], bf16)
            nc.sync.dma_start(id_t[:], ident[:, :])

            # ---- downsample: feat -> q_bf / k_bf  [128, 4096] bf16 (4x scale) ----
            q_bf = qk_pool.tile([C, N], bf16)
            k_bf = qk_pool.tile([C, N], bf16)

            DS_I = 4  # i-rows per chunk
            for src, dst in ((fn, k_bf), (fw, q_bf)):
                src3 = src.ap().rearrange("c (i r) w -> c i (r w)", r=4)  # [128,64,1024]
                dst3 = dst[:].rearrange("c (i w) -> c i w", w=hw)          # [128,64,64]
                for cc in range(hw // DS_I):
                    i0 = cc * DS_I
                    ch = ds_pool.tile([C, DS_I, 512], f32, tag="ch")
                    # rows 4i+1, 4i+2 are adjacent -> 2KB contiguous lines
                    nc.sync.dma_start(ch[:], src3[:, i0 : i0 + DS_I, 256:768])
                    ch5 = ch[:].rearrange("c i (r k f) -> c i r k f", r=2, f=4)
                    # ds = sum of the 4 center samples (scale folded downstream)
                    a = dstmp_pool.tile([C, DS_I, hw], f32, tag="dsa")
                    b = dstmp_pool.tile([C, DS_I, hw], f32, tag="dsb")
                    nc.vector.tensor_tensor(
                        a[:], ch5[:, :, 0, :, 1], ch5[:, :, 1, :, 1], AOp.add
                    )
                    nc.vector.tensor_tensor(
                        b[:], ch5[:, :, 0, :, 2], ch5[:, :, 1, :, 2], AOp.add
                    )
                    nc.vector.tensor_tensor(
                        dst3[:, i0 : i0 + DS_I, :], a[:], b[:], AOp.add
                    )

            # ---- K^T with ones column: kt1 [128(m), 32(mi), 129] bf16 ----
            kt1 = qk_pool.tile([128, MI, 129], bf16)
            nc.vector.memset(kt1[:], 1.0)
            for mi in range(MI):
                pt = ps_t.tile([128, 128], bf16)
                nc.tensor.transpose(pt[:], k_bf[:, mi * 128 : (mi + 1) * 128], id_t[:])
                nc.vector.tensor_copy(kt1[:, mi, 0:128], pt[:])

            # ---- y = W-upsampled attention output [128, 64, 256] bf16 ----
            y = qk_pool.tile([C, hw, HW], bf16)
            y3 = y[:]  # [128, 64, 256]

            # ---- attention: interleave scores of block nb with PV of nb-1 ----
            at_tiles = {}

            def emit_scores(nb, mp):
                """scores^T + exp for m-pair mp of n-block nb."""
                at = at_tiles[nb]
                ps = ps_s.tile([128, 2, 512], f32, tag="ps")
                for s in range(2):
                    mi = 2 * mp + s
                    nc.tensor.matmul(
                        ps[:, s, :],
                        k_bf[:, mi * 128 : (mi + 1) * 128],
                        q_bf[:, nb * 512 : (nb + 1) * 512],
                        start=True,
                        stop=True,
                    )
                nc.scalar.activation(
                    at[:, 2 * mp : 2 * mp + 2, :],
                    ps[:],
                    ActFn.Exp,
                    bias=0.0,
                    scale=EXP_SCALE,
                )

            def emit_pv(nb, ns):
                """PV + normalize + W-up for n-sub-tile ns of n-block nb."""
                at = at_tiles[nb]
                t = nb * NSUB + ns  # global n-tile (2 h-rows)
                po = ps_o.tile([128, 129], f32, tag="po")
                for mi in range(MI):
                    nc.tensor.matmul(
                        po[:],
                        at[:, mi, ns * 128 : (ns + 1) * 128],
                        kt1[:, mi, :],
                        start=(mi == 0),
                        stop=(mi == MI - 1),
                    )
                rcp = small_pool.tile([128, 1], f32, tag="rcp")
                nc.vector.reciprocal(rcp[:], po[:, 128:129])
                ot = small_pool.tile([128, 128], bf16, tag="ot")
                nc.vector.tensor_scalar(ot[:], po[:, 0:128], rcp[:], None, AOp.mult)
                py = ps_y.tile([128, 512], f32, tag="py")
                nc.tensor.matmul(py[:], ot[:], uw_t[:], start=True, stop=True)
                nc.scalar.copy(y3[:, 2 * t : 2 * t + 2, :], py[:])

            for nb in range(NBLK + 1):
                if nb < NBLK:
                    at = attn_pool.tile([128, MI, 512], bf16, tag="at")
                    at_tiles[nb] = at
                # interleave: 4 m-pairs of scores(nb), then one PV sub-tile of nb-1
                for ns in range(NSUB):
                    if nb < NBLK:
                        for mp in range(4 * ns, 4 * ns + 4):
                            emit_scores(nb, mp)
                    if nb > 0:
                        emit_pv(nb - 1, ns)

            # ---- H-upsample + residual, in place on streamed fw row blocks ----
            # out[4k+r] = wa[r]*y[k+d[r]] + wb[r]*y[k+d[r]+1] + fw[4k+r]
            PH = (
                (0.375, 0.625, -1),
                (0.125, 0.875, -1),
                (0.875, 0.125, 0),
                (0.625, 0.375, 0),
            )
            KB = 4  # y rows per block -> 16 output rows
            NKB = hw // KB
            for kb in range(NKB):
                k0 = kb * KB
                h0 = 4 * k0
                fwb = io_pool.tile([C, 4 * KB, HW], f32, tag="io")
                nc.sync.dma_start(fwb[:], fw.ap()[:, h0 : h0 + 4 * KB, :])
                fw4 = fwb[:].rearrange("c (j r) w -> c j r w", r=4)
                for r, (wa, wb, d) in enumerate(PH):
                    js, je = 0, KB
                    if kb == 0 and d == -1:
                        js = 1
                    if kb == NKB - 1 and d == 0:
                        je = KB - 1
                    # edge rows: clamped -> out = 1.0*y[edge] + fw
                    if js == 1:
                        nc.vector.scalar_tensor_tensor(
                            fw4[:, 0, r, :], y3[:, 0, :], 1.0,
                            fw4[:, 0, r, :], AOp.mult, AOp.add,
                        )
                    if je == KB - 1:
                        nc.vector.scalar_tensor_tensor(
                            fw4[:, KB - 1, r, :], y3[:, hw - 1, :], 1.0,
                            fw4[:, KB - 1, r, :], AOp.mult, AOp.add,
                        )
                    cnt = je - js
                    ka = k0 + js + d
                    tm = small_pool.tile([C, KB, HW], bf16, tag="tm")
                    # tm = (wa/wb) * y[ka..] + y[ka+1..]       (all bf16)
                    nc.vector.scalar_tensor_tensor(
                        tm[:, 0:cnt, :], y3[:, ka : ka + cnt, :], wa / wb,
                        y3[:, ka + 1 : ka + 1 + cnt, :], AOp.mult, AOp.add,
                    )
                    # fw_rows += wb * tm   (in place)
                    nc.vector.scalar_tensor_tensor(
                        fw4[:, js:je, r, :], tm[:, 0:cnt, :], wb,
                        fw4[:, js:je, r, :], AOp.mult, AOp.add,
                    )
                nc.sync.dma_start(out.ap()[:, h0 : h0 + 4 * KB, :], fwb[:])

    nc.compile()
    return nc


_NC_CACHE = None


def _get_nc():
    global _NC_CACHE
    if _NC_CACHE is None:
        _NC_CACHE = build_kernel()
    return _NC_CACHE


def run(feat_wide: np.ndarray, feat_narrow: np.ndarray, trace: bool = False):
    """Run on 8 NeuronCores; returns (output [8,128,256,256], BassKernelResults)."""
    from concourse.bass_utils import run_bass_kernel_spmd
    import ml_dtypes

    B, C, H, W = feat_wide.shape
    assert (B, C, H, W) == (8, 128, 256, 256)

    uwblk = _build_uw_block().astype(ml_dtypes.bfloat16)
    identity = np.eye(128, dtype=ml_dtypes.bfloat16)

    nc = _get_nc()
    in_maps = [
        {
            "feat_wide": np.ascontiguousarray(np.asarray(feat_wide[b], dtype=np.float32)),
            "feat_narrow": np.ascontiguousarray(np.asarray(feat_narrow[b], dtype=np.float32)),
            "uwblk": uwblk,
            "ident": identity,
        }
        for b in range(B)
    ]
    res = run_bass_kernel_spmd(nc, in_maps, core_ids=list(range(8)), trace=trace)
    out = np.stack([res.results[b]["out"] for b in range(B)], axis=0)
    return out, res


def kernel(feat_wide: np.ndarray, feat_narrow: np.ndarray) -> np.ndarray:
    out, _ = run(feat_wide, feat_narrow, trace=False)
    return out
